# revision 1
# baseline (speedup 1.0000x reference)
"""Trainium2 Bass kernel for nn_AttentionModel_23304492548756.

Single-launch SPMD design over 8 NeuronCores:
 - 6-layer transformer data-parallel over batch (2 samples/core), weights
   replicated, bf16 matmuls with fp32 PSUM accumulation.
 - The huge end-layer weight We [C*S, O] is sharded over its contraction dim
   (1/8 per core). The relu(h) activations are resharded on-device with an
   AllToAll (each core sends each other core its 64-seq-position slice), then
   each core computes a partial [B, O] which the host sums (cheaper than an
   on-device AllReduce of 32KB).
Layout notes: the residual stream lives as [S, C] fp32 tiles (LayerNorm
reduces over the free dim); matmul stationary operands come from an [C, S]
bf16 transposed copy produced on the PE via transpose-mode.  Attention for
layers >= 1 computes scores directly in [k, q] layout (post-LN scores are
tiny, so exp needs no max subtraction); the softmax denominator comes from a
ones-vector matmul and is folded into the attention-output PSUM drain.
Layer 0 (pre-LN scale ~sqrt(C)) uses a max-subtracted softmax in [q, k]
layout with PE transposes.
"""
import math
from contextlib import ExitStack

import numpy as np
import ml_dtypes

import concourse.bass as bass
import concourse.tile as tile
from concourse import bacc, mybir
from concourse.bass import ts
from concourse.masks import make_identity
from concourse.bass_utils import run_bass_kernel_spmd

FP32 = mybir.dt.float32
BF16 = mybir.dt.bfloat16
AF = mybir.ActivationFunctionType
ALU = mybir.AluOpType

B = 16
C = 512
S = 512
H = 8
L = 6
FF = 2048
O = 512
DH = C // H
EPS = 1e-5
NT = 4           # C/128 = S/128 tiles
NFT = FF // 128  # 16
N_CORES = 8
SPC = B // N_CORES          # samples per core
SSH = S // N_CORES          # seq positions per core in the end layer
KSH = C * S // N_CORES      # end-layer contraction shard
NKT = KSH // 128            # 256 k-tiles in the end layer


def build_full(n_layers=L, n_samples=SPC, n_cores=N_CORES,
               use_g1=False, use_beta1=False, use_g2=False, use_beta2=False,
               use_bo=False, use_b1=False, use_b2=False, emit_hout=False):
    """DRAM inputs (per core):
      h0   [NS, S, C] f32     x^T*sqrt(C)+pe slice (2 samples)
      wq, wk, wv, wo [L, 128, NT*C] bf16   (host-relayout, contiguous lines)
      w1   [L, 128, NT*FF] bf16
      w2   [L, 128, NFT*C] bf16
      we   [KSH, O] bf16      per-core shard of We
      b1   [L, FF] f32 and bo/b2/g1/beta1/g2/beta2 [L, C] f32 when used
    Output: out [B, O] f32 partial (host sums the 8 partials and adds be).
    """
    NL, NS = n_layers, n_samples
    any_vec = (use_g1 or use_beta1 or use_g2 or use_beta2 or use_bo or
               use_b1 or use_b2)
    we_bufs = 1 if any_vec else 3
    nc = bacc.Bacc("TRN2", target_bir_lowering=False, debug=False,
                   num_devices=n_cores)

    h0_d = nc.dram_tensor("h0", [NS, S, C], FP32, kind="ExternalInput").ap()
    wq_d = nc.dram_tensor("wq", [NL, 128, NT * C], BF16, kind="ExternalInput").ap()
    wk_d = nc.dram_tensor("wk", [NL, 128, NT * C], BF16, kind="ExternalInput").ap()
    wv_d = nc.dram_tensor("wv", [NL, 128, NT * C], BF16, kind="ExternalInput").ap()
    wo_d = nc.dram_tensor("wo", [NL, 128, NT * C], BF16, kind="ExternalInput").ap()
    w1_d = nc.dram_tensor("w1", [NL, 128, NT * FF], BF16, kind="ExternalInput").ap()
    w2_d = nc.dram_tensor("w2", [NL, 128, NFT * C], BF16, kind="ExternalInput").ap()
    we_d = nc.dram_tensor("we", [NKT // 4, 128, 4 * O], BF16, kind="ExternalInput").ap()
    b1_d = nc.dram_tensor("b1", [NL, FF], FP32, kind="ExternalInput").ap() if use_b1 else None
    vec_d = {}
    for name, used in (("bo", use_bo), ("b2", use_b2), ("g1", use_g1),
                       ("beta1", use_beta1), ("g2", use_g2), ("beta2", use_beta2)):
        if used:
            vec_d[name] = nc.dram_tensor(name, [NL, C], FP32, kind="ExternalInput").ap()
    out_d = nc.dram_tensor("out", [B, O], FP32, kind="ExternalOutput").ap()
    hout_d = (nc.dram_tensor("hout", [NS, S, C], BF16, kind="ExternalOutput").ap()
              if emit_hout else None)

    with tile.TileContext(nc) as tc, ExitStack() as ctx:
        const_p = ctx.enter_context(tc.tile_pool(name="const", bufs=1))
        wpool = ctx.enter_context(tc.tile_pool(name="w", bufs=2))
        bias_p = ctx.enter_context(tc.tile_pool(name="biasv", bufs=1))
        hsc_p = ctx.enter_context(tc.tile_pool(name="hsc", bufs=8))
        hb_p = ctx.enter_context(tc.tile_pool(name="hb", bufs=4))
        hcs_p = ctx.enter_context(tc.tile_pool(name="hcs", bufs=5))
        qkv_p = ctx.enter_context(tc.tile_pool(name="qkv", bufs=4))
        e_p = ctx.enter_context(tc.tile_pool(name="e", bufs=8))
        at_p = ctx.enter_context(tc.tile_pool(name="at", bufs=8 if not any_vec else 7))
        ot_p = ctx.enter_context(tc.tile_pool(name="ot", bufs=4))
        f1_p = ctx.enter_context(tc.tile_pool(name="f1", bufs=17))
        z_p = ctx.enter_context(tc.tile_pool(name="z", bufs=5))
        rzb_p = ctx.enter_context(tc.tile_pool(name="rzb", bufs=2))
        st_p = ctx.enter_context(tc.tile_pool(name="st", bufs=8))
        out_p = ctx.enter_context(tc.tile_pool(name="out", bufs=2 if not any_vec else 1))
        dram_p = ctx.enter_context(tc.tile_pool(name="dram", bufs=1, space="DRAM"))
        ld_p = ctx.enter_context(tc.tile_pool(name="ld", bufs=1))
        hT_p = ctx.enter_context(tc.tile_pool(name="hT", bufs=8))
        we_p = ctx.enter_context(tc.tile_pool(name="wep", bufs=6))

        ps_big = ctx.enter_context(tc.tile_pool(name="ps_big", bufs=3, space="PSUM"))
        ps_tr = ctx.enter_context(tc.tile_pool(name="ps_tr", bufs=2, space="PSUM"))

        ident = const_p.tile([128, 128], BF16)
        make_identity(nc, ident[:])
        eps_t = const_p.tile([128, 1], FP32)
        nc.vector.memset(eps_t[:], EPS)
        ones_b = const_p.tile([128, 1], BF16)
        nc.vector.memset(ones_b[:], 1.0)
        ones_r = const_p.tile([1, 128], BF16)
        nc.vector.memset(ones_r[:], 1.0)

        a2a_in = dram_p.tile([n_cores, NS, SSH, C], BF16, name="a2a_in")
        a2a_out = dram_p.tile([n_cores, NS, SSH, C], BF16, name="a2a_out")

        hsc = [[None] * NT for _ in range(NS)]
        hcs = [[None] * NT for _ in range(NS)]

        def transpose_to_cs(hb_tiles, tag):
            res = []
            for t in range(NT):
                pst = ps_tr.tile([128, S], BF16, tag="tr", name="tr")
                for u in range(NT):
                    nc.tensor.transpose(pst[:, ts(u, 128)],
                                        hb_tiles[u][:, ts(t, 128)], ident[:])
                dst = hcs_p.tile([128, S], BF16, tag=tag, name=tag)
                nc.vector.tensor_copy(dst[:], pst[:])
                res.append(dst)
            return res

        for s in range(NS):
            hbt = []
            for t in range(NT):
                hsc[s][t] = hsc_p.tile([128, C], FP32, tag="hsc", name="hsc")
                nc.sync.dma_start(hsc[s][t][:], h0_d[s, ts(t, 128), :])
                hb = hb_p.tile([128, C], BF16, tag="hb", name="hb")
                nc.gpsimd.tensor_copy(hb[:], hsc[s][t][:])
                hbt.append(hb)
            hcs[s] = transpose_to_cs(hbt, "hcs")

        for l in range(NL):
            wq_sb = wpool.tile([128, NT, C], BF16, tag="wq", name="wq")
            wk_sb = wpool.tile([128, NT, C], BF16, tag="wk", name="wk")
            wv_sb = wpool.tile([128, NT, C], BF16, tag="wv", name="wv")
            wo_sb = wpool.tile([128, NT, C], BF16, tag="wo", name="wo")
            w1_sb = wpool.tile([128, NT, FF], BF16, tag="w1", name="w1", bufs=1)
            w2_sb = wpool.tile([128, NFT, C], BF16, tag="w2", name="w2", bufs=1)
            nc.sync.dma_start(wq_sb[:], wq_d[l].rearrange("p (ci c) -> p ci c", ci=NT))
            nc.sync.dma_start(wk_sb[:], wk_d[l].rearrange("p (ci c) -> p ci c", ci=NT))
            nc.sync.dma_start(wv_sb[:], wv_d[l].rearrange("p (ci c) -> p ci c", ci=NT))
            nc.sync.dma_start(wo_sb[:], wo_d[l].rearrange("p (ci c) -> p ci c", ci=NT))
            nc.sync.dma_start(w1_sb[:], w1_d[l].rearrange("p (ci f) -> p ci f", ci=NT))
            nc.sync.dma_start(w2_sb[:], w2_d[l].rearrange("p (ft c) -> p ft c", ft=NFT))
            if use_b1:
                b1_sb = bias_p.tile([128, NFT], FP32, tag="b1", name="b1")
                nc.sync.dma_start(b1_sb[:], b1_d[l].rearrange("(ft p) -> p ft", p=128))
            vec_sb = {}
            for name in vec_d:
                vb = bias_p.tile([128, C], FP32, tag=name, name=name)
                src = bass.AP(tensor=vec_d[name].tensor, offset=l * C,
                              ap=[[0, 128], [1, C]])
                nc.gpsimd.dma_start(vb[:], src)
                vec_sb[name] = vb

            for s in range(NS):
                # ---- QKV ----
                qT, kT, vN = [], [], []
                for t in range(NT):
                    psqk = ps_big.tile([128, 2, C], FP32, tag="big", name="big_qk")
                    for ci in range(NT):
                        nc.tensor.matmul(psqk[:, 0, :], wq_sb[:, ci, ts(t, 128)],
                                         hcs[s][ci][:], start=(ci == 0), stop=(ci == NT - 1))
                    for ci in range(NT):
                        nc.tensor.matmul(psqk[:, 1, :], wk_sb[:, ci, ts(t, 128)],
                                         hcs[s][ci][:], start=(ci == 0), stop=(ci == NT - 1))
                    qk = qkv_p.tile([128, 2, S], BF16, tag="qk", name="qk")
                    nc.scalar.copy(qk[:], psqk[:])
                    qT.append(qk[:, 0, :])
                    kT.append(qk[:, 1, :])

                    psv = ps_big.tile([128, 2, C], FP32, tag="big", name="big_v")
                    for ci in range(NT):
                        nc.tensor.matmul(psv[:, 0, :], hcs[s][ci][:, ts(t, 128)],
                                         wv_sb[:, ci, :], start=(ci == 0), stop=(ci == NT - 1))
                    vt = qkv_p.tile([128, C], BF16, tag="v", name="v")
                    nc.vector.tensor_copy(vt[:], psv[:, 0, :])
                    vN.append(vt)

                # ---- attention (head pairs at rows 0-63 / 64-127) ----
                oT = []
                for j in range(NT):
                    if l == 0:
                        # max-subtracted softmax in [q, k] layout + PE transpose
                        E = [[None] * NT for _ in range(2)]
                        for qt in range(NT):
                            zz2 = st_p.tile([128, 2], FP32, tag="zz2", name="zz2")
                            rz2 = st_p.tile([128, 2], FP32, tag="rz2", name="rz2")
                            es = []
                            pssp = ps_big.tile([128, 2, S], FP32, tag="big", name="big_sc0")
                            for sub in range(2):
                                lo = sub * 64
                                nc.tensor.matmul(pssp[:, sub, :],
                                                 qT[j][lo:lo + 64, ts(qt, 128)],
                                                 kT[j][lo:lo + 64, :], start=True, stop=True)
                            for sub in range(2):
                                m = st_p.tile([128, 1], FP32, tag="m", name="m")
                                nc.vector.reduce_max(m[:], pssp[:, sub, :],
                                                     axis=mybir.AxisListType.X)
                                nm = st_p.tile([128, 1], FP32, tag="nm", name="nm")
                                nc.vector.tensor_scalar_mul(nm[:], m[:], -0.125)
                                e = e_p.tile([128, S], BF16, tag="e", name="e", bufs=4)
                                nc.scalar.activation(e[:], pssp[:, sub, :], AF.Exp,
                                                     bias=nm[:], scale=0.125,
                                                     accum_out=zz2[:, sub:sub + 1])
                                es.append(e)
                            nc.vector.reciprocal(rz2[:], zz2[:])
                            for sub in range(2):
                                en = e_p.tile([128, S], BF16, tag="en", name="en", bufs=8 if not any_vec else 7)
                                nc.vector.tensor_scalar_mul(en[:], es[sub][:],
                                                            rz2[:, sub:sub + 1])
                                E[sub][qt] = en
                        AT = [[None] * NT for _ in range(2)]
                        for sub in range(2):
                            for kt2 in range(NT):
                                pst = ps_tr.tile([128, S], BF16, tag="tr", name="tr")
                                for qt in range(NT):
                                    nc.tensor.transpose(pst[:, ts(qt, 128)],
                                                        E[sub][qt][:, ts(kt2, 128)],
                                                        ident[:])
                                at = at_p.tile([128, S], BF16, tag="at", name="at")
                                if (sub + kt2) % 2 == 0:
                                    nc.scalar.copy(at[:], pst[:])
                                else:
                                    nc.vector.tensor_copy(at[:], pst[:])
                                AT[sub][kt2] = at
                        psop = ps_big.tile([128, 2, S], FP32, tag="big", name="big_o0")
                        for kt2 in range(NT):
                            c0 = (2 * j) * DH
                            nc.tensor.matmul(psop[0:64, 0, :], vN[kt2][:, c0:c0 + 64],
                                             AT[0][kt2][:], start=(kt2 == 0),
                                             stop=(kt2 == NT - 1), tile_position=(0, 0))
                            c1 = (2 * j + 1) * DH
                            nc.tensor.matmul(psop[64:128, 1, :], vN[kt2][:, c1:c1 + 64],
                                             AT[1][kt2][:], start=(kt2 == 0),
                                             stop=(kt2 == NT - 1), tile_position=(0, 64))
                        ot = ot_p.tile([128, S], BF16, tag="ot", name="ot")
                        nc.vector.tensor_copy(ot[0:64, :], psop[0:64, 0, :])
                        nc.vector.tensor_copy(ot[64:128, :], psop[64:128, 1, :])
                        oT.append(ot)
                    else:
                        # direct [k, q] scores; Z via ones-matmul; 1/Z bcast on
                        # gpsimd; normalization fused into the oT PSUM drain
                        ET = [[None] * NT for _ in range(2)]
                        for kt2 in range(NT):
                            psSp = ps_big.tile([128, 2, S], FP32, tag="big", name="big_sc")
                            for sub in range(2):
                                lo = sub * 64
                                nc.tensor.matmul(psSp[:, sub, :],
                                                 kT[j][lo:lo + 64, ts(kt2, 128)],
                                                 qT[j][lo:lo + 64, :], start=True, stop=True)
                            ep = e_p.tile([128, 2, S], BF16, tag="ep", name="ep", bufs=5 if not any_vec else 4)
                            nc.scalar.activation(ep[:], psSp[:], AF.Exp, scale=0.125)
                            ET[0][kt2] = ep[:, 0, :]
                            ET[1][kt2] = ep[:, 1, :]
                        rzb = rzb_p.tile([128, S], FP32, tag="rzb", name="rzb")
                        for sub in range(2):
                            psZ = ps_tr.tile([128, S], FP32, tag="tr", name="tr_z")
                            for kt2 in range(NT):
                                nc.tensor.matmul(psZ[0:1, :], ones_b[:],
                                                 ET[sub][kt2][:], start=(kt2 == 0),
                                                 stop=(kt2 == NT - 1))
                            zrow = st_p.tile([1, S], FP32, tag="zrow", name="zrow", bufs=2)
                            nc.scalar.copy(zrow[:], psZ[0:1, :])
                            rz_bf = st_p.tile([1, S], BF16, tag="rzbf", name="rzbf", bufs=2)
                            with nc.allow_low_precision(reason="1/Z bcast via bf16 matmul"):
                                nc.vector.reciprocal(rz_bf[:], zrow[:])
                            psB = ps_tr.tile([128, S], FP32, tag="tr", name="tr_b")
                            nc.tensor.matmul(psB[:], ones_r[:], rz_bf[:],
                                             start=True, stop=True)
                            lo = sub * 64
                            nc.vector.tensor_copy(rzb[lo:lo + 64, :], psB[lo:lo + 64, :])
                        psop = ps_big.tile([128, 2, S], FP32, tag="big", name="big_o")
                        for kt2 in range(NT):
                            c0 = (2 * j) * DH
                            nc.tensor.matmul(psop[0:64, 0, :], vN[kt2][:, c0:c0 + 64],
                                             ET[0][kt2][:], start=(kt2 == 0),
                                             stop=(kt2 == NT - 1), tile_position=(0, 0))
                            c1 = (2 * j + 1) * DH
                            nc.tensor.matmul(psop[64:128, 1, :], vN[kt2][:, c1:c1 + 64],
                                             ET[1][kt2][:], start=(kt2 == 0),
                                             stop=(kt2 == NT - 1), tile_position=(0, 64))
                        ot = ot_p.tile([128, S], BF16, tag="ot", name="ot")
                        nc.vector.tensor_mul(ot[0:64, :], psop[0:64, 0, :], rzb[0:64, :])
                        nc.vector.tensor_mul(ot[64:128, :], psop[64:128, 1, :],
                                             rzb[64:128, :])
                        oT.append(ot)

                # ---- LN over a 4-tile group with batched stats ----
                def ln_phase(ps_tiles, g_sb, beta_sb):
                    zs, hns, hbs = [], [], []
                    rs4 = st_p.tile([128, NT], FP32, tag="rs4", name="rs4")
                    for t in range(NT):
                        z = z_p.tile([128, C], FP32, tag="z", name="z")
                        nc.vector.scalar_tensor_tensor(
                            z[:], ps_tiles[t], 1.0, hsc[s][t][:],
                            op0=ALU.mult, op1=ALU.add,
                            accum_out=rs4[:, t:t + 1])
                        zs.append(z)
                    nm4 = st_p.tile([128, NT], FP32, tag="nm4", name="nm4")
                    nc.vector.tensor_scalar_mul(nm4[:], rs4[:], -1.0 / C)
                    ssq4 = st_p.tile([128, NT], FP32, tag="ssq4", name="ssq4")
                    sq_scr = z_p.tile([128, C], BF16, tag="sqs", name="sqs", bufs=1)
                    for t in range(NT):
                        nc.scalar.activation(sq_scr[:], zs[t][:], AF.Square,
                                             bias=nm4[:, t:t + 1],
                                             accum_out=ssq4[:, t:t + 1])
                    sd4 = st_p.tile([128, NT], FP32, tag="sd4", name="sd4")
                    nc.scalar.activation(sd4[:], ssq4[:], AF.Sqrt, bias=eps_t[:],
                                         scale=1.0 / C)
                    nc.vector.reciprocal(sd4[:], sd4[:])
                    mean4 = st_p.tile([128, NT], FP32, tag="mean4", name="mean4")
                    nc.vector.tensor_scalar_mul(mean4[:], rs4[:], 1.0 / C)
                    for t in range(NT):
                        hn = hsc_p.tile([128, C], FP32, tag="hsc", name="hsc")
                        nc.vector.tensor_scalar(hn[:], zs[t][:],
                                                scalar1=mean4[:, t:t + 1],
                                                scalar2=sd4[:, t:t + 1],
                                                op0=ALU.subtract, op1=ALU.mult)
                        if g_sb is not None:
                            nc.vector.tensor_mul(hn[:], hn[:], g_sb[:])
                        if beta_sb is not None:
                            nc.vector.tensor_add(hn[:], hn[:], beta_sb[:])
                        hb = hb_p.tile([128, C], BF16, tag="hb", name="hb")
                        if g_sb is None and beta_sb is None:
                            nc.gpsimd.tensor_scalar(hb[:], zs[t][:],
                                                    scalar1=mean4[:, t:t + 1],
                                                    scalar2=sd4[:, t:t + 1],
                                                    op0=ALU.subtract, op1=ALU.mult)
                        else:
                            nc.gpsimd.tensor_copy(hb[:], hn[:])
                        hns.append(hn)
                        hbs.append(hb)
                    return hns, hbs

                # ---- attn out proj + residual + LN1 ----
                psa_l = []
                for tp in range(NT // 2):
                    psap = ps_big.tile([128, 2, C], FP32, tag="big", name="big_pr")
                    for half in range(2):
                        t = 2 * tp + half
                        for ci in range(NT):
                            nc.tensor.matmul(psap[:, half, :], oT[ci][:, ts(t, 128)],
                                             wo_sb[:, ci, :], start=(ci == 0),
                                             stop=(ci == NT - 1))
                        if use_bo:
                            nc.vector.tensor_add(psap[:, half, :], psap[:, half, :],
                                                 vec_sb["bo"][:])
                        psa_l.append(psap[:, half, :])
                hns, hb1 = ln_phase(psa_l, vec_sb.get("g1"), vec_sb.get("beta1"))
                hsc[s] = hns
                hcs2 = transpose_to_cs(hb1, "hcs2")

                # ---- FFN ----
                F1 = []
                for fp in range(NFT // 2):
                    ps1p = ps_big.tile([128, 2, S], FP32, tag="big", name="big_f1")
                    for half in range(2):
                        ft = 2 * fp + half
                        for ci in range(NT):
                            nc.tensor.matmul(ps1p[:, half, :],
                                             w1_sb[:, ci, ts(ft, 128)],
                                             hcs2[ci][:], start=(ci == 0),
                                             stop=(ci == NT - 1))
                    f1p = f1_p.tile([128, 2, S], BF16, tag="f1p", name="f1p", bufs=8)
                    if use_b1:
                        for half in range(2):
                            ft = 2 * fp + half
                            nc.scalar.activation(f1p[:, half, :], ps1p[:, half, :],
                                                 AF.Relu, bias=b1_sb[:, ft:ft + 1])
                    else:
                        nc.scalar.activation(f1p[:], ps1p[:], AF.Relu)
                    F1.append(f1p)
                psf_l = []
                for tp in range(NT // 2):
                    psFp = ps_big.tile([128, 2, C], FP32, tag="big", name="big_f2")
                    for half in range(2):
                        t = 2 * tp + half
                        for ft in range(NFT):
                            nc.tensor.matmul(psFp[:, half, :],
                                             F1[ft // 2][:, ft % 2, ts(t, 128)],
                                             w2_sb[:, ft, :], start=(ft == 0),
                                             stop=(ft == NFT - 1))
                        if use_b2:
                            nc.vector.tensor_add(psFp[:, half, :], psFp[:, half, :],
                                                 vec_sb["b2"][:])
                        psf_l.append(psFp[:, half, :])
                hns, hb2 = ln_phase(psf_l, vec_sb.get("g2"), vec_sb.get("beta2"))
                hsc[s] = hns
                if l < NL - 1:
                    hcs[s] = transpose_to_cs(hb2, "hcs")
                else:
                    for t in range(NT):
                        yr = out_p.tile([128, C], BF16, tag="yr", name="yr")
                        nc.scalar.activation(yr[:], hsc[s][t][:], AF.Relu)
                        nc.sync.dma_start(a2a_in[2 * t, s, :, :], yr[0:64, :])
                        nc.sync.dma_start(a2a_in[2 * t + 1, s, :, :], yr[64:128, :])
                        if emit_hout:
                            nc.sync.dma_start(hout_d[s, ts(t, 128), :], yr[:])

        # ======== reshard + end layer ========
        nc.gpsimd.collective_compute(
            "AllToAll", ALU.bypass, replica_groups=[list(range(n_cores))],
            ins=[a2a_in[:]], outs=[a2a_out[:]])

        # hT tiles: [128(k), 16(b)] built by PE transpose of [16, 128] chunks
        NG = NKT // 16                      # 16 groups of 16 k-tiles
        hT = []
        for g in range(NG):
            ld = ld_p.tile([16, 4, C], BF16, tag="ld", name="ld")
            nc.sync.dma_start(ld[:], a2a_out[:, :, g * 4:(g + 1) * 4, :]
                              .rearrange("i b s c -> (i b) s c"))
            pst = ps_tr.tile([128, 16, 16], BF16, tag="tr", name="tr_h")
            for u in range(16):
                nc.tensor.transpose(pst[:, u, :],
                                    ld[:, u // 4, (u % 4) * 128:(u % 4 + 1) * 128],
                                    ident[0:16, 0:16])
            ht = hT_p.tile([128, 16, 16], BF16, tag="hT", name="hT", bufs=16)
            nc.scalar.copy(ht[:], pst[:])
            hT.append(ht)

        psOp = ps_big.tile([128, 2, O], FP32, tag="big", name="big_end")
        psO = psOp[0:B, 0, :]
        for kg in range(NKT // 4):
            we4 = we_p.tile([128, 4, O], BF16, tag="we", name="we", bufs=we_bufs)
            nc.sync.dma_start(we4[:], we_d[kg].rearrange("p (u o) -> p u o", u=4))
            for u in range(4):
                kt = kg * 4 + u
                nc.tensor.matmul(psO, hT[kt // 16][:, kt % 16, :], we4[:, u, :],
                                 start=(kt == 0), stop=(kt == NKT - 1))
        ob = out_p.tile([B, O], FP32, tag="ob", name="ob", bufs=1)
        nc.vector.tensor_copy(ob[:], psO)
        nc.sync.dma_start(out_d[:], ob[:])

    nc.compile()
    return nc


def pe_table():
    pos = np.arange(S, dtype=np.float32)[:, None]
    ie = np.arange(0, C, 2, dtype=np.float32)
    sin = np.sin(pos / 10000.0 ** (2.0 * ie / C))
    cos = np.cos(pos / 10000.0 ** (2.0 * (ie + 1.0) / C))
    pe = np.zeros((S, C), np.float32)
    pe[:, 0::2] = sin
    pe[:, 1::2] = cos
    return pe


_CACHE = {}


def _get_nc(flags):
    if flags not in _CACHE:
        _CACHE[flags] = build_full(
            use_g1=flags[0], use_beta1=flags[1], use_g2=flags[2],
            use_beta2=flags[3], use_bo=flags[4], use_b1=flags[5],
            use_b2=flags[6])
    return _CACHE[flags]


def _bf(a):
    return np.asarray(a).astype(ml_dtypes.bfloat16)


def _relayout(w, inner):
    """[L, n*128, inner] -> [L, 128, n*inner] contiguous per-partition lines."""
    Ln, K, _ = w.shape
    n = K // 128
    return np.ascontiguousarray(
        w.reshape(Ln, n, 128, inner).transpose(0, 2, 1, 3).reshape(Ln, 128, n * inner))


def prep_inputs(x, Wq, Wk, Wv, Wo, bo, g1, beta1, W1, b1, W2, b2, g2, beta2,
                We, be):
    x = np.asarray(x, dtype=np.float32)
    h0 = (np.swapaxes(x, 1, 2) * math.sqrt(C) + pe_table()[None]).astype(np.float32)

    bo, b1, b2 = (np.asarray(a, np.float32) for a in (bo, b1, b2))
    g1, beta1 = (np.asarray(a, np.float32) for a in (g1, beta1))
    g2, beta2 = (np.asarray(a, np.float32) for a in (g2, beta2))
    flags = (bool((g1 != 1).any()), bool(beta1.any()), bool((g2 != 1).any()),
             bool(beta2.any()), bool(bo.any()), bool(b1.any()), bool(b2.any()))

    We_bf = _bf(We)
    base = {"wq": _relayout(_bf(Wq), C), "wk": _relayout(_bf(Wk), C),
            "wv": _relayout(_bf(Wv), C), "wo": _relayout(_bf(Wo), C),
            "w1": _relayout(_bf(W1), FF), "w2": _relayout(_bf(W2), C)}
    names = ("g1", "beta1", "g2", "beta2", "bo", "b1", "b2")
    vals = (g1, beta1, g2, beta2, bo, b1, b2)
    for nm, used, val in zip(names, flags, vals):
        if used:
            base[nm] = val
    in_maps = []
    for c in range(N_CORES):
        m = dict(base)
        m["h0"] = h0[c * SPC:(c + 1) * SPC]
        wsh = We_bf[c * KSH:(c + 1) * KSH]
        m["we"] = np.ascontiguousarray(
            wsh.reshape(NKT // 4, 4, 128, O).transpose(0, 2, 1, 3)
               .reshape(NKT // 4, 128, 4 * O))
        in_maps.append(m)
    return flags, in_maps


def kernel(x, Wq, Wk, Wv, Wo, bo, g1, beta1, W1, b1, W2, b2, g2, beta2, We,
           be, **_unused):
    flags, in_maps = prep_inputs(x, Wq, Wk, Wv, Wo, bo, g1, beta1, W1, b1,
                                 W2, b2, g2, beta2, We, be)
    nc = _get_nc(flags)
    res = run_bass_kernel_spmd(nc, in_maps, list(range(N_CORES)))
    out = np.zeros((B, O), np.float32)
    for c in range(N_CORES):
        out += res.results[c]["out"]
    out += np.asarray(be, np.float32)[None, :]
    return out



# revision 23
# speedup vs baseline: 1.2642x; 1.2642x over previous
"""Trainium2 Bass kernel for nn_AttentionModel_23304492548756.

Single-launch SPMD design over 8 NeuronCores:
 - 6-layer transformer data-parallel over batch (2 samples/core), weights
   replicated, bf16 matmuls with fp32 PSUM accumulation.
 - The huge end-layer weight We [C*S, O] is sharded over its contraction dim
   (1/8 per core). The relu(h) activations are resharded on-device with an
   AllToAll (each core sends each other core its 64-seq-position slice), then
   each core computes a partial [B, O] which the host sums (cheaper than an
   on-device AllReduce of 32KB).
Layout notes: the residual stream lives as [S, C] fp32 tiles (LayerNorm
reduces over the free dim); matmul stationary operands come from an [C, S]
bf16 transposed copy produced on the PE via transpose-mode.  Attention for
layers >= 1 computes scores directly in [k, q] layout (post-LN scores are
tiny, so exp needs no max subtraction); the softmax denominator comes for
free from a ones-column appended to the V stationary (PSUM row 64), its
reciprocal is computed as exp(-ln(Z)) on ACT (reads PSUM natively),
broadcast along partitions by a [1,64]-ones PE matmul, and folded into the
attention-output PSUM drain (the odd head is normalized into a base-0
staging tile and partition-shifted by DMA, since DVE lanes are
partition-locked).  Layer 0 (pre-LN scale ~sqrt(C)) uses a max-subtracted
softmax in [q, k] layout with PE transposes.
Engine notes: LayerNorm uses raw-moment stats (var = E[z^2]-E[z]^2) so the
Square accumulation never waits on the mean; per-tile stat scalars live at
16-byte strides (dual-PTR tensor_scalar hits a ~20x slow path at offsets
mod 16 not in {0,4}); nothing elementwise runs on GpSimd (Q7 ucode is ~10x
slower than DVE); and the ACT table set is pinned to
natural_log_exp_and_others at compile time — every ACT function used
(Exp, Ln, Square, Relu, Copy, Identity) lives in that one set, so no
~1.3us table reloads (1/sqrt is exp(-0.5*ln(x)), avoiding Sqrt's set).
"""
import math
from contextlib import ExitStack

import numpy as np
import ml_dtypes

import concourse.bass as bass
import concourse.tile as tile
from concourse import bacc, mybir
from concourse.bass import ts
from concourse.masks import make_identity
from concourse.bass_utils import run_bass_kernel_spmd

FP32 = mybir.dt.float32
BF16 = mybir.dt.bfloat16
AF = mybir.ActivationFunctionType
ALU = mybir.AluOpType

B = 16
C = 512
S = 512
H = 8
L = 6
FF = 2048
O = 512
DH = C // H
EPS = 1e-5
NT = 4           # C/128 = S/128 tiles
NFT = FF // 128  # 16
N_CORES = 8
SPC = B // N_CORES          # samples per core
SSH = S // N_CORES          # seq positions per core in the end layer
KSH = C * S // N_CORES      # end-layer contraction shard
NKT = KSH // 128            # 256 k-tiles in the end layer


def build_full(n_layers=L, n_samples=SPC, n_cores=N_CORES,
               use_g1=False, use_beta1=False, use_g2=False, use_beta2=False,
               use_bo=False, use_b1=False, use_b2=False, emit_hout=False):
    """DRAM inputs (per core):
      h0   [NS, S, C] f32     x^T*sqrt(C)+pe slice (2 samples)
      wq, wk, wv, wo [L, 128, NT*C] bf16   (host-relayout, contiguous lines)
      w1   [L, 128, NT*FF] bf16
      w2   [L, 128, NFT*C] bf16
      we   [KSH, O] bf16      per-core shard of We
      b1   [L, FF] f32 and bo/b2/g1/beta1/g2/beta2 [L, C] f32 when used
    Output: out [B, O] f32 partial (host sums the 8 partials and adds be).
    """
    NL, NS = n_layers, n_samples
    any_vec = (use_g1 or use_beta1 or use_g2 or use_beta2 or use_bo or
               use_b1 or use_b2)
    we_bufs = 1 if any_vec else 2
    nc = bacc.Bacc("TRN2", target_bir_lowering=False, debug=False,
                   num_devices=n_cores)

    h0_d = nc.dram_tensor("h0", [NS, S, C], FP32, kind="ExternalInput").ap()
    wq_d = nc.dram_tensor("wq", [NL, 128, NT * C], BF16, kind="ExternalInput").ap()
    wk_d = nc.dram_tensor("wk", [NL, 128, NT * C], BF16, kind="ExternalInput").ap()
    wv_d = nc.dram_tensor("wv", [NL, 128, NT * C], BF16, kind="ExternalInput").ap()
    wo_d = nc.dram_tensor("wo", [NL, 128, NT * C], BF16, kind="ExternalInput").ap()
    w1_d = nc.dram_tensor("w1", [NL, 128, NT * FF], BF16, kind="ExternalInput").ap()
    w2_d = nc.dram_tensor("w2", [NL, 128, NFT * C], BF16, kind="ExternalInput").ap()
    we_d = nc.dram_tensor("we", [NKT // 4, 128, 4 * O], BF16, kind="ExternalInput").ap()
    b1_d = nc.dram_tensor("b1", [NL, FF], FP32, kind="ExternalInput").ap() if use_b1 else None
    vec_d = {}
    for name, used in (("bo", use_bo), ("b2", use_b2), ("g1", use_g1),
                       ("beta1", use_beta1), ("g2", use_g2), ("beta2", use_beta2)):
        if used:
            vec_d[name] = nc.dram_tensor(name, [NL, C], FP32, kind="ExternalInput").ap()
    out_d = nc.dram_tensor("out", [B, O], FP32, kind="ExternalOutput").ap()
    hout_d = (nc.dram_tensor("hout", [NS, S, C], BF16, kind="ExternalOutput").ap()
              if emit_hout else None)

    with tile.TileContext(nc) as tc, ExitStack() as ctx:
        const_p = ctx.enter_context(tc.tile_pool(name="const", bufs=1))
        wpool = ctx.enter_context(tc.tile_pool(name="w", bufs=2))
        bias_p = ctx.enter_context(tc.tile_pool(name="biasv", bufs=1))
        hsc_p = ctx.enter_context(tc.tile_pool(name="hsc", bufs=8))
        hb_p = ctx.enter_context(tc.tile_pool(name="hb", bufs=4))
        hcs_p = ctx.enter_context(tc.tile_pool(name="hcs", bufs=5))
        qkv_p = ctx.enter_context(tc.tile_pool(name="qkv", bufs=4))
        e_p = ctx.enter_context(tc.tile_pool(name="e", bufs=8))
        at_p = ctx.enter_context(tc.tile_pool(name="at", bufs=8 if not any_vec else 7))
        ot_p = ctx.enter_context(tc.tile_pool(name="ot", bufs=4))
        f1_p = ctx.enter_context(tc.tile_pool(name="f1", bufs=17))
        z_p = ctx.enter_context(tc.tile_pool(name="z", bufs=5))
        rzb_p = ctx.enter_context(tc.tile_pool(name="rzb", bufs=2))
        st_p = ctx.enter_context(tc.tile_pool(name="st", bufs=8))
        out_p = ctx.enter_context(tc.tile_pool(name="out", bufs=2 if not any_vec else 1))
        dram_p = ctx.enter_context(tc.tile_pool(name="dram", bufs=1, space="DRAM"))
        ld_p = ctx.enter_context(tc.tile_pool(name="ld", bufs=1))
        hT_p = ctx.enter_context(tc.tile_pool(name="hT", bufs=8))
        we_p = ctx.enter_context(tc.tile_pool(name="wep", bufs=3))

        # PSUM: pool A holds 2-bank [128, 2, 512] f32 tiles; pool B holds
        # 1-bank tiles (attention-out 65-row tiles, transposes, end psO).
        ps_big = ctx.enter_context(tc.tile_pool(name="ps_big", bufs=2, space="PSUM"))
        ps_sm = ctx.enter_context(tc.tile_pool(name="ps_sm", bufs=4, space="PSUM"))

        ident = const_p.tile([128, 128], BF16)
        make_identity(nc, ident[:])
        eps_t = const_p.tile([128, 1], FP32)
        nc.vector.memset(eps_t[:], EPS)
        ones_dh = const_p.tile([1, DH], BF16)
        nc.vector.memset(ones_dh[:], 1.0)

        a2a_in = dram_p.tile([n_cores, NS, SSH, C], BF16, name="a2a_in")
        a2a_out = dram_p.tile([n_cores, NS, SSH, C], BF16, name="a2a_out")

        hsc = [[None] * NT for _ in range(NS)]
        hcs = [[None] * NT for _ in range(NS)]

        def transpose_to_cs(hb_tiles, tag):
            res = []
            for t in range(NT):
                pst = ps_sm.tile([128, S], BF16, tag="sm", name="tr")
                for u in range(NT):
                    nc.tensor.transpose(pst[:, ts(u, 128)],
                                        hb_tiles[u][:, ts(t, 128)], ident[:])
                dst = hcs_p.tile([128, S], BF16, tag=tag, name=tag)
                nc.vector.tensor_copy(dst[:], pst[:])
                res.append(dst)
            return res

        for s in range(NS):
            hbt = []
            for t in range(NT):
                hsc[s][t] = hsc_p.tile([128, C], FP32, tag="hsc", name="hsc")
                nc.sync.dma_start(hsc[s][t][:], h0_d[s, ts(t, 128), :])
                hb = hb_p.tile([128, C], BF16, tag="hb", name="hb")
                if t % 2 == 0:
                    nc.vector.tensor_copy(hb[:], hsc[s][t][:])
                else:
                    nc.scalar.copy(hb[:], hsc[s][t][:])
                hbt.append(hb)
            hcs[s] = transpose_to_cs(hbt, "hcs")

        for l in range(NL):
            wq_sb = wpool.tile([128, NT, C], BF16, tag="wq", name="wq")
            wk_sb = wpool.tile([128, NT, C], BF16, tag="wk", name="wk")
            wv_sb = wpool.tile([128, NT, C], BF16, tag="wv", name="wv")
            wo_sb = wpool.tile([128, NT, C], BF16, tag="wo", name="wo")
            w1_sb = wpool.tile([128, NT, FF], BF16, tag="w1", name="w1", bufs=1)
            w2_sb = wpool.tile([128, NFT, C], BF16, tag="w2", name="w2", bufs=1)
            nc.sync.dma_start(wq_sb[:], wq_d[l].rearrange("p (ci c) -> p ci c", ci=NT))
            nc.sync.dma_start(wk_sb[:], wk_d[l].rearrange("p (ci c) -> p ci c", ci=NT))
            nc.sync.dma_start(wv_sb[:], wv_d[l].rearrange("p (ci c) -> p ci c", ci=NT))
            nc.sync.dma_start(wo_sb[:], wo_d[l].rearrange("p (ci c) -> p ci c", ci=NT))
            nc.sync.dma_start(w1_sb[:], w1_d[l].rearrange("p (ci f) -> p ci f", ci=NT))
            nc.sync.dma_start(w2_sb[:], w2_d[l].rearrange("p (ft c) -> p ft c", ft=NFT))
            if use_b1:
                b1_sb = bias_p.tile([128, NFT], FP32, tag="b1", name="b1")
                nc.sync.dma_start(b1_sb[:], b1_d[l].rearrange("(ft p) -> p ft", p=128))
            vec_sb = {}
            for name in vec_d:
                vb = bias_p.tile([128, C], FP32, tag=name, name=name)
                src = bass.AP(tensor=vec_d[name].tensor, offset=l * C,
                              ap=[[0, 128], [1, C]])
                nc.gpsimd.dma_start(vb[:], src)
                vec_sb[name] = vb

            for s in range(NS):
                # ---- QKV ----
                qT, kT, vN = [], [], []
                for t in range(NT):
                    psqk = ps_big.tile([128, 2, C], FP32, tag="big", name="big_qk")
                    for ci in range(NT):
                        nc.tensor.matmul(psqk[:, 0, :], wq_sb[:, ci, ts(t, 128)],
                                         hcs[s][ci][:], start=(ci == 0), stop=(ci == NT - 1))
                    for ci in range(NT):
                        nc.tensor.matmul(psqk[:, 1, :], wk_sb[:, ci, ts(t, 128)],
                                         hcs[s][ci][:], start=(ci == 0), stop=(ci == NT - 1))
                    qk = qkv_p.tile([128, 2, S], BF16, tag="qk", name="qk")
                    if t % 2 == 0:
                        nc.scalar.copy(qk[:], psqk[:])
                    else:
                        nc.vector.tensor_copy(qk[:], psqk[:])
                    qT.append(qk[:, 0, :])
                    kT.append(qk[:, 1, :])

                    psv = ps_big.tile([128, 2, C], FP32, tag="big", name="big_v")
                    for ci in range(NT):
                        nc.tensor.matmul(psv[:, 0, :], hcs[s][ci][:, ts(t, 128)],
                                         wv_sb[:, ci, :], start=(ci == 0), stop=(ci == NT - 1))
                    # v tile with a ones column per head: [128, h, 0:64] = V,
                    # [:, h, 64] = 1 so the attention-output matmul also
                    # produces the softmax denominator Z in PSUM row 64.
                    vt = qkv_p.tile([128, H, DH + 1], BF16, tag="v", name="v")
                    nc.vector.memset(vt[:, :, DH:DH + 1], 1.0)
                    nc.vector.tensor_copy(
                        vt[:, :, 0:DH],
                        psv[:, 0, :].rearrange("p (h d) -> p h d", h=H))
                    vN.append(vt)

                # ---- attention (head pairs at rows 0-63 / 64-127) ----
                oT = []
                for j in range(NT):
                    if l == 0:
                        # max-subtracted softmax in [q, k] layout + PE transpose
                        E = [[None] * NT for _ in range(2)]
                        for qt in range(NT):
                            zz2 = st_p.tile([128, 2], FP32, tag="zz2", name="zz2")
                            rz2 = st_p.tile([128, 2], FP32, tag="rz2", name="rz2")
                            es = []
                            pssp = ps_big.tile([128, 2, S], FP32, tag="big", name="big_sc0")
                            for sub in range(2):
                                lo = sub * 64
                                nc.tensor.matmul(pssp[:, sub, :],
                                                 qT[j][lo:lo + 64, ts(qt, 128)],
                                                 kT[j][lo:lo + 64, :], start=True, stop=True)
                            for sub in range(2):
                                m = st_p.tile([128, 1], FP32, tag="m", name="m")
                                nc.vector.reduce_max(m[:], pssp[:, sub, :],
                                                     axis=mybir.AxisListType.X)
                                nm = st_p.tile([128, 1], FP32, tag="nm", name="nm")
                                nc.vector.tensor_scalar_mul(nm[:], m[:], -0.125)
                                e = e_p.tile([128, S], BF16, tag="e", name="e", bufs=4)
                                nc.scalar.activation(e[:], pssp[:, sub, :], AF.Exp,
                                                     bias=nm[:], scale=0.125,
                                                     accum_out=zz2[:, sub:sub + 1])
                                es.append(e)
                            nc.vector.reciprocal(rz2[:], zz2[:])
                            for sub in range(2):
                                en = e_p.tile([128, S], BF16, tag="en", name="en", bufs=8 if not any_vec else 7)
                                nc.vector.tensor_scalar_mul(en[:], es[sub][:],
                                                            rz2[:, sub:sub + 1])
                                E[sub][qt] = en
                        AT = [[None] * NT for _ in range(2)]
                        for sub in range(2):
                            for kt2 in range(NT):
                                pst = ps_sm.tile([128, S], BF16, tag="sm", name="tr")
                                for qt in range(NT):
                                    nc.tensor.transpose(pst[:, ts(qt, 128)],
                                                        E[sub][qt][:, ts(kt2, 128)],
                                                        ident[:])
                                at = at_p.tile([128, S], BF16, tag="at", name="at")
                                if (sub + kt2) % 2 == 0:
                                    nc.scalar.copy(at[:], pst[:])
                                else:
                                    nc.vector.tensor_copy(at[:], pst[:])
                                AT[sub][kt2] = at
                        psop = ps_big.tile([128, 2, S], FP32, tag="big", name="big_o0")
                        for kt2 in range(NT):
                            c0 = 2 * j
                            nc.tensor.matmul(psop[0:64, 0, :], vN[kt2][:, c0, 0:DH],
                                             AT[0][kt2][:], start=(kt2 == 0),
                                             stop=(kt2 == NT - 1), tile_position=(0, 0))
                            c1 = 2 * j + 1
                            nc.tensor.matmul(psop[64:128, 1, :], vN[kt2][:, c1, 0:DH],
                                             AT[1][kt2][:], start=(kt2 == 0),
                                             stop=(kt2 == NT - 1), tile_position=(0, 64))
                        ot = ot_p.tile([128, S], BF16, tag="ot", name="ot")
                        nc.vector.tensor_copy(ot[0:64, :], psop[0:64, 0, :])
                        nc.vector.tensor_copy(ot[64:128, :], psop[64:128, 1, :])
                        oT.append(ot)
                    else:
                        # direct [k, q] scores; Z arrives free in PSUM row 64
                        # via the ones column of vt; 1/Z (fast approx) is
                        # partition-broadcast by a stride-0 DMA and folded
                        # into the attention-output PSUM drain.
                        ET = [[None] * NT for _ in range(2)]
                        for kt2 in range(NT):
                            psSp = ps_big.tile([128, 2, S], FP32, tag="big", name="big_sc")
                            for sub in range(2):
                                lo = sub * 64
                                nc.tensor.matmul(psSp[:, sub, :],
                                                 kT[j][lo:lo + 64, ts(kt2, 128)],
                                                 qT[j][lo:lo + 64, :], start=True, stop=True)
                            ep = e_p.tile([128, 2, S], BF16, tag="ep", name="ep", bufs=5 if not any_vec else 4)
                            nc.scalar.activation(ep[:], psSp[:], AF.Exp, scale=0.125)
                            ET[0][kt2] = ep[:, 0, :]
                            ET[1][kt2] = ep[:, 1, :]
                        ot = ot_p.tile([128, S], BF16, tag="ot", name="ot")
                        for sub in range(2):
                            pso = ps_sm.tile([DH + 1, S], FP32, tag="sm", name="ps_o")
                            hd = 2 * j + sub
                            for kt2 in range(NT):
                                nc.tensor.matmul(pso[:], vN[kt2][:, hd, :],
                                                 ET[sub][kt2][:], start=(kt2 == 0),
                                                 stop=(kt2 == NT - 1))
                            # 1/Z = exp(-ln(Z)) on ACT (reads PSUM natively;
                            # ln/exp share the pinned table set)
                            rz = st_p.tile([1, S], FP32, tag="rz", name="rz", bufs=2)
                            nc.scalar.activation(rz[:], pso[DH:DH + 1, :], AF.Ln)
                            rz_bf = st_p.tile([1, S], BF16, tag="rzbf",
                                              name="rzbf", bufs=2)
                            nc.scalar.activation(rz_bf[:], rz[:], AF.Exp,
                                                 scale=-1.0)
                            # broadcast 1/Z along partitions on the PE
                            psB = ps_sm.tile([DH, S], FP32, tag="sm", name="ps_b")
                            nc.tensor.matmul(psB[:], ones_dh[:], rz_bf[:],
                                             start=True, stop=True)
                            rzb = rzb_p.tile([DH, S], BF16, tag="rzb", name="rzb")
                            nc.vector.tensor_copy(rzb[:], psB[:])
                            # DVE lanes are partition-locked, so the odd head
                            # is normalized into a base-0 staging tile and
                            # partition-shifted to rows 64-127 by DMA.
                            if sub == 0:
                                nc.vector.tensor_mul(ot[0:DH, :], pso[0:DH, :],
                                                     rzb[:])
                            else:
                                ot1 = ot_p.tile([DH, S], BF16, tag="ot1",
                                                name="ot1", bufs=2)
                                nc.vector.tensor_mul(ot1[:], pso[0:DH, :], rzb[:])
                                nc.sync.dma_start(ot[DH:128, :], ot1[:])
                        oT.append(ot)

                # ---- LN over a 4-tile group with batched stats ----
                # stats tiles are [128, NT, 4] f32 so each per-tile scalar
                # column sits at a 16-byte stride (dual-PTR tensor_scalar
                # falls off a cliff when the scalar pointer is at offset
                # mod 16 not in {0, 4}).
                def ln_phase(ps_tiles, g_sb, beta_sb):
                    # raw-moment stats: z[t] immediately feeds Square (no wait
                    # on the mean), var = ssq/C - mean^2, rstd via ln/exp.
                    # hb (bf16, feeds the PE transposes = the critical path)
                    # is produced directly from z on ACT/DVE with per-tile
                    # scale/bias, in parallel with the fp32 residual hn.
                    zs, hns, hbs = [], [], []
                    rs4 = st_p.tile([128, NT, 4], FP32, tag="rs4", name="rs4")
                    ssq4 = st_p.tile([128, NT, 4], FP32, tag="ssq4", name="ssq4")
                    sq_scr = z_p.tile([128, C], BF16, tag="sqs", name="sqs", bufs=1)
                    for t in range(NT):
                        z = z_p.tile([128, C], FP32, tag="z", name="z")
                        nc.vector.scalar_tensor_tensor(
                            z[:], ps_tiles[t], 1.0, hsc[s][t][:],
                            op0=ALU.mult, op1=ALU.add,
                            accum_out=rs4[:, t, 0:1])
                        nc.scalar.activation(sq_scr[:], z[:], AF.Square,
                                             accum_out=ssq4[:, t, 0:1])
                        zs.append(z)
                    mean4 = st_p.tile([128, NT, 4], FP32, tag="mean4", name="mean4")
                    nc.vector.tensor_scalar_mul(mean4[:, :, 0:1], rs4[:, :, 0:1],
                                                1.0 / C)
                    msq4 = st_p.tile([128, NT, 4], FP32, tag="msq4", name="msq4")
                    nc.vector.tensor_mul(msq4[:, :, 0:1], mean4[:, :, 0:1],
                                         mean4[:, :, 0:1])
                    varg = st_p.tile([128, NT, 4], FP32, tag="varg", name="varg")
                    nc.vector.tensor_scalar(varg[:, :, 0:1], ssq4[:, :, 0:1],
                                            scalar1=1.0 / C, scalar2=EPS,
                                            op0=ALU.mult, op1=ALU.add)
                    nc.vector.tensor_sub(varg[:, :, 0:1], varg[:, :, 0:1],
                                         msq4[:, :, 0:1])
                    # 1/sqrt(v) = exp(-0.5*ln(v)): stays inside the pinned
                    # ACT table set (Sqrt would force a table reload).
                    sd4 = st_p.tile([128, NT, 4], FP32, tag="sd4", name="sd4")
                    nc.scalar.activation(sd4[:, :, 0:1], varg[:, :, 0:1], AF.Ln)
                    nc.scalar.activation(sd4[:, :, 0:1], sd4[:, :, 0:1], AF.Exp,
                                         scale=-0.5)
                    nmr4 = st_p.tile([128, NT, 4], FP32, tag="nmr4", name="nmr4")
                    nc.vector.tensor_mul(nmr4[:, :, 0:1], mean4[:, :, 0:1],
                                         sd4[:, :, 0:1])
                    nc.vector.tensor_scalar_mul(nmr4[:, :, 0:1], nmr4[:, :, 0:1],
                                                -1.0)
                    for t in range(NT):
                        hb = hb_p.tile([128, C], BF16, tag="hb", name="hb")
                        if t % 2 == 0:
                            # hb = z*rstd + (-mean*rstd) on ACT
                            nc.scalar.activation(hb[:], zs[t][:], AF.Identity,
                                                 bias=nmr4[:, t, 0:1],
                                                 scale=sd4[:, t, 0:1])
                        else:
                            nc.vector.tensor_scalar(hb[:], zs[t][:],
                                                    scalar1=mean4[:, t, 0:1],
                                                    scalar2=sd4[:, t, 0:1],
                                                    op0=ALU.subtract,
                                                    op1=ALU.mult)
                        hbs.append(hb)
                    for t in range(NT):
                        hn = hsc_p.tile([128, C], FP32, tag="hsc", name="hsc")
                        nc.vector.tensor_scalar(hn[:], zs[t][:],
                                                scalar1=mean4[:, t, 0:1],
                                                scalar2=sd4[:, t, 0:1],
                                                op0=ALU.subtract, op1=ALU.mult)
                        if g_sb is not None:
                            nc.vector.tensor_mul(hn[:], hn[:], g_sb[:])
                        if beta_sb is not None:
                            nc.vector.tensor_add(hn[:], hn[:], beta_sb[:])
                        if g_sb is not None or beta_sb is not None:
                            nc.vector.tensor_copy(hbs[t][:], hn[:])
                        hns.append(hn)
                    return hns, hbs

                # ---- attn out proj + residual + LN1 ----
                psa_l = []
                for tp in range(NT // 2):
                    psap = ps_big.tile([128, 2, C], FP32, tag="big", name="big_pr")
                    for half in range(2):
                        t = 2 * tp + half
                        for ci in range(NT):
                            nc.tensor.matmul(psap[:, half, :], oT[ci][:, ts(t, 128)],
                                             wo_sb[:, ci, :], start=(ci == 0),
                                             stop=(ci == NT - 1))
                        if use_bo:
                            nc.vector.tensor_add(psap[:, half, :], psap[:, half, :],
                                                 vec_sb["bo"][:])
                        psa_l.append(psap[:, half, :])
                hns, hb1 = ln_phase(psa_l, vec_sb.get("g1"), vec_sb.get("beta1"))
                hsc[s] = hns
                hcs2 = transpose_to_cs(hb1, "hcs2")

                # ---- FFN ----
                F1 = []
                for fp in range(NFT // 2):
                    ps1p = ps_big.tile([128, 2, S], FP32, tag="big", name="big_f1")
                    for half in range(2):
                        ft = 2 * fp + half
                        for ci in range(NT):
                            nc.tensor.matmul(ps1p[:, half, :],
                                             w1_sb[:, ci, ts(ft, 128)],
                                             hcs2[ci][:], start=(ci == 0),
                                             stop=(ci == NT - 1))
                    f1p = f1_p.tile([128, 2, S], BF16, tag="f1p", name="f1p", bufs=8)
                    if use_b1:
                        for half in range(2):
                            ft = 2 * fp + half
                            nc.scalar.activation(f1p[:, half, :], ps1p[:, half, :],
                                                 AF.Relu, bias=b1_sb[:, ft:ft + 1])
                    else:
                        nc.scalar.activation(f1p[:], ps1p[:], AF.Relu)
                    F1.append(f1p)
                psf_l = []
                for tp in range(NT // 2):
                    psFp = ps_big.tile([128, 2, C], FP32, tag="big", name="big_f2")
                    for half in range(2):
                        t = 2 * tp + half
                        for ft in range(NFT):
                            nc.tensor.matmul(psFp[:, half, :],
                                             F1[ft // 2][:, ft % 2, ts(t, 128)],
                                             w2_sb[:, ft, :], start=(ft == 0),
                                             stop=(ft == NFT - 1))
                        if use_b2:
                            nc.vector.tensor_add(psFp[:, half, :], psFp[:, half, :],
                                                 vec_sb["b2"][:])
                        psf_l.append(psFp[:, half, :])
                hns, hb2 = ln_phase(psf_l, vec_sb.get("g2"), vec_sb.get("beta2"))
                hsc[s] = hns
                if l < NL - 1:
                    hcs[s] = transpose_to_cs(hb2, "hcs")
                else:
                    for t in range(NT):
                        yr = out_p.tile([128, C], BF16, tag="yr", name="yr")
                        nc.scalar.activation(yr[:], hsc[s][t][:], AF.Relu)
                        nc.sync.dma_start(a2a_in[2 * t, s, :, :], yr[0:64, :])
                        nc.sync.dma_start(a2a_in[2 * t + 1, s, :, :], yr[64:128, :])
                        if emit_hout:
                            nc.sync.dma_start(hout_d[s, ts(t, 128), :], yr[:])

        # ======== reshard + end layer ========
        nc.gpsimd.collective_compute(
            "AllToAll", ALU.bypass, replica_groups=[list(range(n_cores))],
            ins=[a2a_in[:]], outs=[a2a_out[:]])

        # hT tiles: [128(k), 16(b)] built by PE transpose of [16, 128] chunks
        NG = NKT // 16                      # 16 groups of 16 k-tiles
        hT = []
        for g in range(NG):
            ld = ld_p.tile([16, 4, C], BF16, tag="ld", name="ld")
            nc.sync.dma_start(ld[:], a2a_out[:, :, g * 4:(g + 1) * 4, :]
                              .rearrange("i b s c -> (i b) s c"))
            pst = ps_sm.tile([128, 16, 16], BF16, tag="sm", name="tr_h")
            for u in range(16):
                nc.tensor.transpose(pst[:, u, :],
                                    ld[:, u // 4, (u % 4) * 128:(u % 4 + 1) * 128],
                                    ident[0:16, 0:16])
            ht = hT_p.tile([128, 16, 16], BF16, tag="hT", name="hT", bufs=16)
            nc.scalar.copy(ht[:], pst[:])
            hT.append(ht)

        psOp = ps_sm.tile([B, O], FP32, tag="sm", name="ps_end")
        psO = psOp[:]
        for kg in range(NKT // 4):
            we4 = we_p.tile([128, 4, O], BF16, tag="we", name="we", bufs=we_bufs)
            nc.sync.dma_start(we4[:], we_d[kg].rearrange("p (u o) -> p u o", u=4))
            for u in range(4):
                kt = kg * 4 + u
                nc.tensor.matmul(psO, hT[kt // 16][:, kt % 16, :], we4[:, u, :],
                                 start=(kt == 0), stop=(kt == NKT - 1))
        ob = out_p.tile([B, O], FP32, tag="ob", name="ob", bufs=1)
        nc.vector.tensor_copy(ob[:], psO)
        nc.sync.dma_start(out_d[:], ob[:])

    _compile_with_pinned_act_set(nc)
    return nc


def _compile_with_pinned_act_set(nc):
    """Compile with the ACT table chooser restricted to
    natural_log_exp_and_others (covers every ACT function this kernel uses:
    Exp, Ln, Square, Relu, Copy, Identity). The default first-match chooser
    alternates exp_and_others / natural_log on the Exp<->Ln boundary, paying
    a ~1.3us table reload ~4x per layer. Positions are preserved so
    act_func_set_id stays aligned with act_info.json; the patch is restored
    immediately after compile."""
    import concourse.bacc as bacc_mod
    orig = bacc_mod.get_activation_tables

    def pinned(arch):
        return {name: (funcs if name == "natural_log_exp_and_others" else set())
                for name, funcs in orig(arch).items()}

    bacc_mod.get_activation_tables = pinned
    try:
        nc.compile()
    finally:
        bacc_mod.get_activation_tables = orig


def pe_table():
    pos = np.arange(S, dtype=np.float32)[:, None]
    ie = np.arange(0, C, 2, dtype=np.float32)
    sin = np.sin(pos / 10000.0 ** (2.0 * ie / C))
    cos = np.cos(pos / 10000.0 ** (2.0 * (ie + 1.0) / C))
    pe = np.zeros((S, C), np.float32)
    pe[:, 0::2] = sin
    pe[:, 1::2] = cos
    return pe


_CACHE = {}


def _get_nc(flags):
    if flags not in _CACHE:
        _CACHE[flags] = build_full(
            use_g1=flags[0], use_beta1=flags[1], use_g2=flags[2],
            use_beta2=flags[3], use_bo=flags[4], use_b1=flags[5],
            use_b2=flags[6])
    return _CACHE[flags]


def _bf(a):
    return np.asarray(a).astype(ml_dtypes.bfloat16)


def _relayout(w, inner):
    """[L, n*128, inner] -> [L, 128, n*inner] contiguous per-partition lines."""
    Ln, K, _ = w.shape
    n = K // 128
    return np.ascontiguousarray(
        w.reshape(Ln, n, 128, inner).transpose(0, 2, 1, 3).reshape(Ln, 128, n * inner))


def prep_inputs(x, Wq, Wk, Wv, Wo, bo, g1, beta1, W1, b1, W2, b2, g2, beta2,
                We, be):
    x = np.asarray(x, dtype=np.float32)
    h0 = (np.swapaxes(x, 1, 2) * math.sqrt(C) + pe_table()[None]).astype(np.float32)

    bo, b1, b2 = (np.asarray(a, np.float32) for a in (bo, b1, b2))
    g1, beta1 = (np.asarray(a, np.float32) for a in (g1, beta1))
    g2, beta2 = (np.asarray(a, np.float32) for a in (g2, beta2))
    flags = (bool((g1 != 1).any()), bool(beta1.any()), bool((g2 != 1).any()),
             bool(beta2.any()), bool(bo.any()), bool(b1.any()), bool(b2.any()))

    We_bf = _bf(We)
    base = {"wq": _relayout(_bf(Wq), C), "wk": _relayout(_bf(Wk), C),
            "wv": _relayout(_bf(Wv), C), "wo": _relayout(_bf(Wo), C),
            "w1": _relayout(_bf(W1), FF), "w2": _relayout(_bf(W2), C)}
    names = ("g1", "beta1", "g2", "beta2", "bo", "b1", "b2")
    vals = (g1, beta1, g2, beta2, bo, b1, b2)
    for nm, used, val in zip(names, flags, vals):
        if used:
            base[nm] = val
    in_maps = []
    for c in range(N_CORES):
        m = dict(base)
        m["h0"] = h0[c * SPC:(c + 1) * SPC]
        wsh = We_bf[c * KSH:(c + 1) * KSH]
        m["we"] = np.ascontiguousarray(
            wsh.reshape(NKT // 4, 4, 128, O).transpose(0, 2, 1, 3)
               .reshape(NKT // 4, 128, 4 * O))
        in_maps.append(m)
    return flags, in_maps


def kernel(x, Wq, Wk, Wv, Wo, bo, g1, beta1, W1, b1, W2, b2, g2, beta2, We,
           be, **_unused):
    flags, in_maps = prep_inputs(x, Wq, Wk, Wv, Wo, bo, g1, beta1, W1, b1,
                                 W2, b2, g2, beta2, We, be)
    nc = _get_nc(flags)
    res = run_bass_kernel_spmd(nc, in_maps, list(range(N_CORES)))
    out = np.zeros((B, O), np.float32)
    for c in range(N_CORES):
        out += res.results[c]["out"]
    out += np.asarray(be, np.float32)[None, :]
    return out


# revision 27
# speedup vs baseline: 1.3638x; 1.0788x over previous
"""Trainium2 Bass kernel for nn_AttentionModel_23304492548756.

Single-launch SPMD design over 8 NeuronCores:
 - 6-layer transformer data-parallel over batch (2 samples/core), weights
   replicated, bf16 matmuls with fp32 PSUM accumulation.
 - The huge end-layer weight We [C*S, O] is sharded over its contraction dim
   (1/8 per core). The relu(h) activations are resharded on-device with an
   AllToAll (each core sends each other core its 64-seq-position slice), then
   each core computes a partial [B, O] which the host sums (cheaper than an
   on-device AllReduce of 32KB).
Layout notes: the residual stream lives as [S, C] fp32 tiles (LayerNorm
reduces over the free dim); matmul stationary operands come from an [C, S]
bf16 transposed copy produced on the PE via transpose-mode.  Attention for
layers >= 1 computes scores directly in [k, q] layout (post-LN scores are
tiny, so exp needs no max subtraction); the softmax denominator comes for
free from a ones-column appended to the V stationary (PSUM row 64), its
reciprocal is computed as exp(-ln(Z)) on ACT (reads PSUM natively),
broadcast along partitions by a [1,64]-ones PE matmul, and folded into the
attention-output PSUM drain (the odd head is normalized into a base-0
staging tile and partition-shifted by DMA, since DVE lanes are
partition-locked).  Layer 0 (pre-LN scale ~sqrt(C)) uses a max-subtracted
softmax in [q, k] layout with PE transposes.
Engine notes: LayerNorm uses raw-moment stats (var = E[z^2]-E[z]^2) so the
Square accumulation never waits on the mean; per-tile stat scalars live at
16-byte strides (dual-PTR tensor_scalar hits a ~20x slow path at offsets
mod 16 not in {0,4}); nothing elementwise runs on GpSimd (Q7 ucode is ~10x
slower than DVE); and the ACT table set is pinned to
natural_log_exp_and_others at compile time — every ACT function used
(Exp, Ln, Square, Relu, Copy, Identity) lives in that one set, so no
~1.3us table reloads (1/sqrt is exp(-0.5*ln(x)), avoiding Sqrt's set).
"""
import math
from contextlib import ExitStack

import numpy as np
import ml_dtypes

import concourse.bass as bass
import concourse.tile as tile
from concourse import bacc, mybir
from concourse.bass import ts
from concourse.masks import make_identity
from concourse.bass_utils import run_bass_kernel_spmd

FP32 = mybir.dt.float32
BF16 = mybir.dt.bfloat16
AF = mybir.ActivationFunctionType
ALU = mybir.AluOpType

B = 16
C = 512
S = 512
H = 8
L = 6
FF = 2048
O = 512
DH = C // H
EPS = 1e-5
NT = 4           # C/128 = S/128 tiles
NFT = FF // 128  # 16
N_CORES = 8
SPC = B // N_CORES          # samples per core
SSH = S // N_CORES          # seq positions per core in the end layer
KSH = C * S // N_CORES      # end-layer contraction shard
NKT = KSH // 128            # 256 k-tiles in the end layer


def build_full(n_layers=L, n_samples=SPC, n_cores=N_CORES,
               use_g1=False, use_beta1=False, use_g2=False, use_beta2=False,
               use_bo=False, use_b1=False, use_b2=False, emit_hout=False):
    """DRAM inputs (per core):
      h0   [NS, S, C] f32     x^T*sqrt(C)+pe slice (2 samples)
      wq, wk, wv, wo [L, 128, NT*C] bf16   (host-relayout, contiguous lines)
      w1   [L, 128, NT*FF] bf16
      w2   [L, 128, NFT*C] bf16
      we   [KSH, O] bf16      per-core shard of We
      b1   [L, FF] f32 and bo/b2/g1/beta1/g2/beta2 [L, C] f32 when used
    Output: out [B, O] f32 partial (host sums the 8 partials and adds be).
    """
    NL, NS = n_layers, n_samples
    any_vec = (use_g1 or use_beta1 or use_g2 or use_beta2 or use_bo or
               use_b1 or use_b2)
    we_bufs = 1 if any_vec else 2
    nc = bacc.Bacc("TRN2", target_bir_lowering=False, debug=False,
                   num_devices=n_cores)

    h0_d = nc.dram_tensor("h0", [NS, S, C], FP32, kind="ExternalInput").ap()
    wq_d = nc.dram_tensor("wq", [NL, 128, NT * C], BF16, kind="ExternalInput").ap()
    wk_d = nc.dram_tensor("wk", [NL, 128, NT * C], BF16, kind="ExternalInput").ap()
    wv_d = nc.dram_tensor("wv", [NL, 128, NT * C], BF16, kind="ExternalInput").ap()
    wo_d = nc.dram_tensor("wo", [NL, 128, NT * C], BF16, kind="ExternalInput").ap()
    w1_d = nc.dram_tensor("w1", [NL, 128, NT * FF], BF16, kind="ExternalInput").ap()
    w2_d = nc.dram_tensor("w2", [NL, 128, NFT * C], BF16, kind="ExternalInput").ap()
    we_d = nc.dram_tensor("we", [NKT // 4, 128, 4 * O], BF16, kind="ExternalInput").ap()
    b1_d = nc.dram_tensor("b1", [NL, FF], FP32, kind="ExternalInput").ap() if use_b1 else None
    vec_d = {}
    for name, used in (("bo", use_bo), ("b2", use_b2), ("g1", use_g1),
                       ("beta1", use_beta1), ("g2", use_g2), ("beta2", use_beta2)):
        if used:
            vec_d[name] = nc.dram_tensor(name, [NL, C], FP32, kind="ExternalInput").ap()
    out_d = nc.dram_tensor("out", [B, O], FP32, kind="ExternalOutput").ap()
    hout_d = (nc.dram_tensor("hout", [NS, S, C], BF16, kind="ExternalOutput").ap()
              if emit_hout else None)

    with tile.TileContext(nc) as tc, ExitStack() as ctx:
        const_p = ctx.enter_context(tc.tile_pool(name="const", bufs=1))
        wpool = ctx.enter_context(tc.tile_pool(name="w", bufs=2))
        bias_p = ctx.enter_context(tc.tile_pool(name="biasv", bufs=1))
        hsc_p = ctx.enter_context(tc.tile_pool(name="hsc", bufs=8))
        hb_p = ctx.enter_context(tc.tile_pool(name="hb", bufs=4))
        hcs_p = ctx.enter_context(tc.tile_pool(name="hcs", bufs=5))
        qkv_p = ctx.enter_context(tc.tile_pool(name="qkv", bufs=4))
        e_p = ctx.enter_context(tc.tile_pool(name="e", bufs=8))
        at_p = ctx.enter_context(tc.tile_pool(name="at", bufs=8 if not any_vec else 7))
        ot_p = ctx.enter_context(tc.tile_pool(name="ot", bufs=4))
        f1_p = ctx.enter_context(tc.tile_pool(name="f1", bufs=17))
        z_p = ctx.enter_context(tc.tile_pool(name="z", bufs=5))
        rzb_p = ctx.enter_context(tc.tile_pool(name="rzb", bufs=2))
        st_p = ctx.enter_context(tc.tile_pool(name="st", bufs=8))
        out_p = ctx.enter_context(tc.tile_pool(name="out", bufs=2 if not any_vec else 1))
        dram_p = ctx.enter_context(tc.tile_pool(name="dram", bufs=1, space="DRAM"))
        ld_p = ctx.enter_context(tc.tile_pool(name="ld", bufs=1))
        hT_p = ctx.enter_context(tc.tile_pool(name="hT", bufs=8))
        we_p = ctx.enter_context(tc.tile_pool(name="wep", bufs=3))

        # PSUM: pool A holds 2-bank [128, 2, 512] f32 tiles; pool B holds
        # 1-bank tiles (attention-out 65-row tiles, transposes, end psO).
        ps_big = ctx.enter_context(tc.tile_pool(name="ps_big", bufs=2, space="PSUM"))
        ps_sm = ctx.enter_context(tc.tile_pool(name="ps_sm", bufs=4, space="PSUM"))

        ident = const_p.tile([128, 128], BF16)
        make_identity(nc, ident[:])
        eps_t = const_p.tile([128, 1], FP32)
        nc.vector.memset(eps_t[:], EPS)
        ones_dh = const_p.tile([1, DH], BF16)
        nc.vector.memset(ones_dh[:], 1.0)

        a2a_in = dram_p.tile([NS, n_cores, SSH, C], BF16, name="a2a_in")
        a2a_out = dram_p.tile([NS, n_cores, SSH, C], BF16, name="a2a_out")

        hsc = [[None] * NT for _ in range(NS)]
        hcs = [[None] * NT for _ in range(NS)]

        def transpose_to_cs(hb_tiles, tag, bufs=5):
            res = []
            for t in range(NT):
                pst = ps_sm.tile([128, S], BF16, tag="sm", name="tr")
                for u in range(NT):
                    nc.tensor.transpose(pst[:, ts(u, 128)],
                                        hb_tiles[u][:, ts(t, 128)], ident[:])
                dst = hcs_p.tile([128, S], BF16, tag=tag, name=tag, bufs=bufs)
                if t % 2 == 0:
                    nc.vector.tensor_copy(dst[:], pst[:])
                else:
                    nc.scalar.copy(dst[:], pst[:])
                res.append(dst)
            return res

        for s in range(NS):
            hbt = []
            for t in range(NT):
                hsc[s][t] = hsc_p.tile([128, C], FP32, tag="hsc", name="hsc")
                nc.sync.dma_start(hsc[s][t][:], h0_d[s, ts(t, 128), :])
                hb = hb_p.tile([128, C], BF16, tag="hb", name="hb")
                if t % 2 == 0:
                    nc.vector.tensor_copy(hb[:], hsc[s][t][:])
                else:
                    nc.scalar.copy(hb[:], hsc[s][t][:])
                hbt.append(hb)
            hcs[s] = transpose_to_cs(hbt, "hcs", bufs=8)

        for l in range(NL):
            wq_sb = wpool.tile([128, NT, C], BF16, tag="wq", name="wq")
            wk_sb = wpool.tile([128, NT, C], BF16, tag="wk", name="wk")
            wv_sb = wpool.tile([128, NT, C], BF16, tag="wv", name="wv")
            wo_sb = wpool.tile([128, NT, C], BF16, tag="wo", name="wo")
            w1_sb = wpool.tile([128, NT, FF], BF16, tag="w1", name="w1", bufs=1)
            w2_sb = wpool.tile([128, NFT, C], BF16, tag="w2", name="w2", bufs=1)
            nc.sync.dma_start(wq_sb[:], wq_d[l].rearrange("p (ci c) -> p ci c", ci=NT))
            nc.sync.dma_start(wk_sb[:], wk_d[l].rearrange("p (ci c) -> p ci c", ci=NT))
            nc.sync.dma_start(wv_sb[:], wv_d[l].rearrange("p (ci c) -> p ci c", ci=NT))
            nc.sync.dma_start(wo_sb[:], wo_d[l].rearrange("p (ci c) -> p ci c", ci=NT))
            nc.sync.dma_start(w1_sb[:], w1_d[l].rearrange("p (ci f) -> p ci f", ci=NT))
            nc.sync.dma_start(w2_sb[:], w2_d[l].rearrange("p (ft c) -> p ft c", ft=NFT))
            if use_b1:
                b1_sb = bias_p.tile([128, NFT], FP32, tag="b1", name="b1")
                nc.sync.dma_start(b1_sb[:], b1_d[l].rearrange("(ft p) -> p ft", p=128))
            vec_sb = {}
            for name in vec_d:
                vb = bias_p.tile([128, C], FP32, tag=name, name=name)
                src = bass.AP(tensor=vec_d[name].tensor, offset=l * C,
                              ap=[[0, 128], [1, C]])
                nc.gpsimd.dma_start(vb[:], src)
                vec_sb[name] = vb

            hcs2_s = [None] * NS
            for s in range(NS):
                # ---- QKV ----
                qT, kT, vN = [], [], []
                for t in range(NT):
                    psqk = ps_big.tile([128, 2, C], FP32, tag="big", name="big_qk")
                    for ci in range(NT):
                        nc.tensor.matmul(psqk[:, 0, :], wq_sb[:, ci, ts(t, 128)],
                                         hcs[s][ci][:], start=(ci == 0), stop=(ci == NT - 1))
                    for ci in range(NT):
                        nc.tensor.matmul(psqk[:, 1, :], wk_sb[:, ci, ts(t, 128)],
                                         hcs[s][ci][:], start=(ci == 0), stop=(ci == NT - 1))
                    qk = qkv_p.tile([128, 2, S], BF16, tag="qk", name="qk")
                    if t % 2 == 0:
                        nc.scalar.copy(qk[:], psqk[:])
                    else:
                        nc.vector.tensor_copy(qk[:], psqk[:])
                    qT.append(qk[:, 0, :])
                    kT.append(qk[:, 1, :])

                    psv = ps_big.tile([128, 2, C], FP32, tag="big", name="big_v")
                    for ci in range(NT):
                        nc.tensor.matmul(psv[:, 0, :], hcs[s][ci][:, ts(t, 128)],
                                         wv_sb[:, ci, :], start=(ci == 0), stop=(ci == NT - 1))
                    # v tile with a ones column per head: [128, h, 0:64] = V,
                    # [:, h, 64] = 1 so the attention-output matmul also
                    # produces the softmax denominator Z in PSUM row 64.
                    vt = qkv_p.tile([128, H, DH + 1], BF16, tag="v", name="v")
                    nc.vector.memset(vt[:, :, DH:DH + 1], 1.0)
                    nc.vector.tensor_copy(
                        vt[:, :, 0:DH],
                        psv[:, 0, :].rearrange("p (h d) -> p h d", h=H))
                    vN.append(vt)

                # ---- attention (head pairs at rows 0-63 / 64-127) ----
                oT = []
                for j in range(NT):
                    if l == 0:
                        # max-subtracted softmax in [q, k] layout + PE transpose
                        E = [[None] * NT for _ in range(2)]
                        for qt in range(NT):
                            zz2 = st_p.tile([128, 2], FP32, tag="zz2", name="zz2")
                            rz2 = st_p.tile([128, 2], FP32, tag="rz2", name="rz2")
                            es = []
                            pssp = ps_big.tile([128, 2, S], FP32, tag="big", name="big_sc0")
                            for sub in range(2):
                                lo = sub * 64
                                nc.tensor.matmul(pssp[:, sub, :],
                                                 qT[j][lo:lo + 64, ts(qt, 128)],
                                                 kT[j][lo:lo + 64, :], start=True, stop=True)
                            for sub in range(2):
                                m = st_p.tile([128, 1], FP32, tag="m", name="m")
                                nc.vector.reduce_max(m[:], pssp[:, sub, :],
                                                     axis=mybir.AxisListType.X)
                                nm = st_p.tile([128, 1], FP32, tag="nm", name="nm")
                                nc.vector.tensor_scalar_mul(nm[:], m[:], -0.125)
                                e = e_p.tile([128, S], BF16, tag="e", name="e", bufs=4)
                                nc.scalar.activation(e[:], pssp[:, sub, :], AF.Exp,
                                                     bias=nm[:], scale=0.125,
                                                     accum_out=zz2[:, sub:sub + 1])
                                es.append(e)
                            nc.vector.reciprocal(rz2[:], zz2[:])
                            for sub in range(2):
                                en = e_p.tile([128, S], BF16, tag="en", name="en", bufs=8 if not any_vec else 7)
                                nc.vector.tensor_scalar_mul(en[:], es[sub][:],
                                                            rz2[:, sub:sub + 1])
                                E[sub][qt] = en
                        AT = [[None] * NT for _ in range(2)]
                        for sub in range(2):
                            for kt2 in range(NT):
                                pst = ps_sm.tile([128, S], BF16, tag="sm", name="tr")
                                for qt in range(NT):
                                    nc.tensor.transpose(pst[:, ts(qt, 128)],
                                                        E[sub][qt][:, ts(kt2, 128)],
                                                        ident[:])
                                at = f1_p.tile([128, S], BF16, tag="f1p", name="at", bufs=8)
                                if (sub + kt2) % 2 == 0:
                                    nc.scalar.copy(at[:], pst[:])
                                else:
                                    nc.vector.tensor_copy(at[:], pst[:])
                                AT[sub][kt2] = at
                        psop = ps_big.tile([128, 2, S], FP32, tag="big", name="big_o0")
                        for kt2 in range(NT):
                            c0 = 2 * j
                            nc.tensor.matmul(psop[0:64, 0, :], vN[kt2][:, c0, 0:DH],
                                             AT[0][kt2][:], start=(kt2 == 0),
                                             stop=(kt2 == NT - 1), tile_position=(0, 0))
                            c1 = 2 * j + 1
                            nc.tensor.matmul(psop[64:128, 1, :], vN[kt2][:, c1, 0:DH],
                                             AT[1][kt2][:], start=(kt2 == 0),
                                             stop=(kt2 == NT - 1), tile_position=(0, 64))
                        ot = ot_p.tile([128, S], BF16, tag="ot", name="ot")
                        nc.vector.tensor_copy(ot[0:64, :], psop[0:64, 0, :])
                        nc.vector.tensor_copy(ot[64:128, :], psop[64:128, 1, :])
                        oT.append(ot)
                    else:
                        # direct [k, q] scores; Z arrives free in PSUM row 64
                        # via the ones column of vt; 1/Z (fast approx) is
                        # partition-broadcast by a stride-0 DMA and folded
                        # into the attention-output PSUM drain.
                        ET = [[None] * NT for _ in range(2)]
                        for kt2 in range(NT):
                            psSp = ps_big.tile([128, 2, S], FP32, tag="big", name="big_sc")
                            for sub in range(2):
                                lo = sub * 64
                                nc.tensor.matmul(psSp[:, sub, :],
                                                 kT[j][lo:lo + 64, ts(kt2, 128)],
                                                 qT[j][lo:lo + 64, :], start=True, stop=True)
                            ep = e_p.tile([128, 2, S], BF16, tag="ep", name="ep", bufs=5 if not any_vec else 4)
                            nc.scalar.activation(ep[:], psSp[:], AF.Exp, scale=0.125)
                            ET[0][kt2] = ep[:, 0, :]
                            ET[1][kt2] = ep[:, 1, :]
                        ot = ot_p.tile([128, S], BF16, tag="ot", name="ot")
                        for sub in range(2):
                            pso = ps_sm.tile([DH + 1, S], FP32, tag="sm", name="ps_o")
                            hd = 2 * j + sub
                            for kt2 in range(NT):
                                nc.tensor.matmul(pso[:], vN[kt2][:, hd, :],
                                                 ET[sub][kt2][:], start=(kt2 == 0),
                                                 stop=(kt2 == NT - 1))
                            # 1/Z = exp(-ln(Z)) on ACT (reads PSUM natively;
                            # ln/exp share the pinned table set)
                            rz = st_p.tile([1, S], FP32, tag="rz", name="rz", bufs=2)
                            nc.scalar.activation(rz[:], pso[DH:DH + 1, :], AF.Ln)
                            rz_bf = st_p.tile([1, S], BF16, tag="rzbf",
                                              name="rzbf", bufs=2)
                            nc.scalar.activation(rz_bf[:], rz[:], AF.Exp,
                                                 scale=-1.0)
                            # broadcast 1/Z along partitions on the PE
                            psB = ps_sm.tile([DH, S], FP32, tag="sm", name="ps_b")
                            nc.tensor.matmul(psB[:], ones_dh[:], rz_bf[:],
                                             start=True, stop=True)
                            rzb = rzb_p.tile([DH, S], BF16, tag="rzb", name="rzb")
                            nc.vector.tensor_copy(rzb[:], psB[:])
                            # DVE lanes are partition-locked, so the odd head
                            # is normalized into a base-0 staging tile and
                            # partition-shifted to rows 64-127 by DMA.
                            if sub == 0:
                                nc.vector.tensor_mul(ot[0:DH, :], pso[0:DH, :],
                                                     rzb[:])
                            else:
                                ot1 = ot_p.tile([DH, S], BF16, tag="ot1",
                                                name="ot1", bufs=2)
                                nc.vector.tensor_mul(ot1[:], pso[0:DH, :], rzb[:])
                                nc.gpsimd.dma_start(ot[DH:128, :], ot1[:])
                        oT.append(ot)

                # ---- LN over a 4-tile group with batched stats ----
                # stats tiles are [128, NT, 4] f32 so each per-tile scalar
                # column sits at a 16-byte stride (dual-PTR tensor_scalar
                # falls off a cliff when the scalar pointer is at offset
                # mod 16 not in {0, 4}).
                def ln_phase(s, ps_tiles, g_sb, beta_sb):
                    # raw-moment stats: z[t] immediately feeds Square (no wait
                    # on the mean), var = ssq/C - mean^2, rstd via ln/exp.
                    # hb (bf16, feeds the PE transposes = the critical path)
                    # is produced directly from z on ACT/DVE with per-tile
                    # scale/bias, in parallel with the fp32 residual hn.
                    zs, hns, hbs = [], [], []
                    rs4 = st_p.tile([128, NT, 4], FP32, tag="rs4", name="rs4")
                    ssq4 = st_p.tile([128, NT, 4], FP32, tag="ssq4", name="ssq4")
                    sq_scr = z_p.tile([128, C], BF16, tag="sqs", name="sqs", bufs=1)
                    for t in range(NT):
                        z = z_p.tile([128, C], FP32, tag="z", name="z")
                        nc.vector.scalar_tensor_tensor(
                            z[:], ps_tiles[t], 1.0, hsc[s][t][:],
                            op0=ALU.mult, op1=ALU.add,
                            accum_out=rs4[:, t, 0:1])
                        nc.scalar.activation(sq_scr[:], z[:], AF.Square,
                                             accum_out=ssq4[:, t, 0:1])
                        zs.append(z)
                    mean4 = st_p.tile([128, NT, 4], FP32, tag="mean4", name="mean4")
                    nc.vector.tensor_scalar_mul(mean4[:, :, 0:1], rs4[:, :, 0:1],
                                                1.0 / C)
                    msq4 = st_p.tile([128, NT, 4], FP32, tag="msq4", name="msq4")
                    nc.vector.tensor_mul(msq4[:, :, 0:1], mean4[:, :, 0:1],
                                         mean4[:, :, 0:1])
                    varg = st_p.tile([128, NT, 4], FP32, tag="varg", name="varg")
                    nc.vector.tensor_scalar(varg[:, :, 0:1], ssq4[:, :, 0:1],
                                            scalar1=1.0 / C, scalar2=EPS,
                                            op0=ALU.mult, op1=ALU.add)
                    nc.vector.tensor_sub(varg[:, :, 0:1], varg[:, :, 0:1],
                                         msq4[:, :, 0:1])
                    # 1/sqrt(v) = exp(-0.5*ln(v)): stays inside the pinned
                    # ACT table set (Sqrt would force a table reload).
                    sd4 = st_p.tile([128, NT, 4], FP32, tag="sd4", name="sd4")
                    nc.scalar.activation(sd4[:, :, 0:1], varg[:, :, 0:1], AF.Ln)
                    nc.scalar.activation(sd4[:, :, 0:1], sd4[:, :, 0:1], AF.Exp,
                                         scale=-0.5)
                    nmr4 = st_p.tile([128, NT, 4], FP32, tag="nmr4", name="nmr4")
                    nc.vector.tensor_mul(nmr4[:, :, 0:1], mean4[:, :, 0:1],
                                         sd4[:, :, 0:1])
                    nc.vector.tensor_scalar_mul(nmr4[:, :, 0:1], nmr4[:, :, 0:1],
                                                -1.0)
                    for t in range(NT):
                        hb = hb_p.tile([128, C], BF16, tag="hb", name="hb")
                        if t % 2 == 0:
                            # hb = z*rstd + (-mean*rstd) on ACT
                            nc.scalar.activation(hb[:], zs[t][:], AF.Identity,
                                                 bias=nmr4[:, t, 0:1],
                                                 scale=sd4[:, t, 0:1])
                        else:
                            nc.vector.tensor_scalar(hb[:], zs[t][:],
                                                    scalar1=mean4[:, t, 0:1],
                                                    scalar2=sd4[:, t, 0:1],
                                                    op0=ALU.subtract,
                                                    op1=ALU.mult)
                        hbs.append(hb)
                    for t in range(NT):
                        hn = hsc_p.tile([128, C], FP32, tag="hsc", name="hsc")
                        nc.vector.tensor_scalar(hn[:], zs[t][:],
                                                scalar1=mean4[:, t, 0:1],
                                                scalar2=sd4[:, t, 0:1],
                                                op0=ALU.subtract, op1=ALU.mult)
                        if g_sb is not None:
                            nc.vector.tensor_mul(hn[:], hn[:], g_sb[:])
                        if beta_sb is not None:
                            nc.vector.tensor_add(hn[:], hn[:], beta_sb[:])
                        if g_sb is not None or beta_sb is not None:
                            nc.vector.tensor_copy(hbs[t][:], hn[:])
                        hns.append(hn)
                    return hns, hbs

                # ---- attn out proj + residual + LN1 ----
                psa_l = []
                for tp in range(NT // 2):
                    psap = ps_big.tile([128, 2, C], FP32, tag="big", name="big_pr")
                    for half in range(2):
                        t = 2 * tp + half
                        for ci in range(NT):
                            nc.tensor.matmul(psap[:, half, :], oT[ci][:, ts(t, 128)],
                                             wo_sb[:, ci, :], start=(ci == 0),
                                             stop=(ci == NT - 1))
                        if use_bo:
                            nc.vector.tensor_add(psap[:, half, :], psap[:, half, :],
                                                 vec_sb["bo"][:])
                        psa_l.append(psap[:, half, :])
                hns, hb1 = ln_phase(s, psa_l, vec_sb.get("g1"), vec_sb.get("beta1"))
                hsc[s] = hns
                hcs2_s[s] = transpose_to_cs(hb1, "hcs2", bufs=8)

            # ---- FFN: second pass over samples, so one sample's matmuls
            # fill the other sample's LayerNorm-chain stalls ----
            for s in range(NS):
                hcs2 = hcs2_s[s]
                F1 = []
                for fp in range(NFT // 2):
                    ps1p = ps_big.tile([128, 2, S], FP32, tag="big", name="big_f1")
                    for half in range(2):
                        ft = 2 * fp + half
                        for ci in range(NT):
                            nc.tensor.matmul(ps1p[:, half, :],
                                             w1_sb[:, ci, ts(ft, 128)],
                                             hcs2[ci][:], start=(ci == 0),
                                             stop=(ci == NT - 1))
                    f1p = f1_p.tile([128, 2, S], BF16, tag="f1p", name="f1p", bufs=8)
                    if use_b1:
                        for half in range(2):
                            ft = 2 * fp + half
                            nc.scalar.activation(f1p[:, half, :], ps1p[:, half, :],
                                                 AF.Relu, bias=b1_sb[:, ft:ft + 1])
                    else:
                        nc.scalar.activation(f1p[:], ps1p[:], AF.Relu)
                    F1.append(f1p)
                psf_l = []
                for tp in range(NT // 2):
                    psFp = ps_big.tile([128, 2, C], FP32, tag="big", name="big_f2")
                    for half in range(2):
                        t = 2 * tp + half
                        for ft in range(NFT):
                            nc.tensor.matmul(psFp[:, half, :],
                                             F1[ft // 2][:, ft % 2, ts(t, 128)],
                                             w2_sb[:, ft, :], start=(ft == 0),
                                             stop=(ft == NFT - 1))
                        if use_b2:
                            nc.vector.tensor_add(psFp[:, half, :], psFp[:, half, :],
                                                 vec_sb["b2"][:])
                        psf_l.append(psFp[:, half, :])
                hns, hb2 = ln_phase(s, psf_l, vec_sb.get("g2"), vec_sb.get("beta2"))
                hsc[s] = hns
                if l < NL - 1:
                    hcs[s] = transpose_to_cs(hb2, "hcs", bufs=8)
                else:
                    for t in range(NT):
                        yr = out_p.tile([128, C], BF16, tag="yr", name="yr")
                        nc.scalar.activation(yr[:], hsc[s][t][:], AF.Relu)
                        nc.sync.dma_start(a2a_in[s, 2 * t, :, :], yr[0:64, :])
                        nc.sync.dma_start(a2a_in[s, 2 * t + 1, :, :], yr[64:128, :])
                        if emit_hout:
                            nc.sync.dma_start(hout_d[s, ts(t, 128), :], yr[:])

        # ======== reshard + end layer ========
        for s in range(NS):
            nc.gpsimd.collective_compute(
                "AllToAll", ALU.bypass, replica_groups=[list(range(n_cores))],
                ins=[a2a_in[s]], outs=[a2a_out[s]])

        # hT tiles: [128(k), 16(b)] built by PE transpose of [16, 128] chunks
        NG = NKT // 16                      # 16 groups of 16 k-tiles
        hT = []
        for g in range(NG):
            ld = ld_p.tile([16, 4, C], BF16, tag="ld", name="ld")
            nc.sync.dma_start(ld[:], a2a_out[:, :, g * 4:(g + 1) * 4, :]
                              .rearrange("b i s c -> (b i) s c"))
            pst = ps_sm.tile([128, 16, 16], BF16, tag="sm", name="tr_h")
            for u in range(16):
                nc.tensor.transpose(pst[:, u, :],
                                    ld[:, u // 4, (u % 4) * 128:(u % 4 + 1) * 128],
                                    ident[0:16, 0:16])
            ht = hT_p.tile([128, 16, 16], BF16, tag="hT", name="hT", bufs=16)
            nc.scalar.copy(ht[:], pst[:])
            hT.append(ht)

        psOp = ps_sm.tile([B, O], FP32, tag="sm", name="ps_end")
        psO = psOp[:]
        for kg in range(NKT // 4):
            we4 = we_p.tile([128, 4, O], BF16, tag="we", name="we", bufs=we_bufs)
            nc.sync.dma_start(we4[:], we_d[kg].rearrange("p (u o) -> p u o", u=4))
            for u in range(4):
                kt = kg * 4 + u
                nc.tensor.matmul(psO, hT[kt // 16][:, kt % 16, :], we4[:, u, :],
                                 start=(kt == 0), stop=(kt == NKT - 1))
        ob = out_p.tile([B, O], FP32, tag="ob", name="ob", bufs=1)
        nc.vector.tensor_copy(ob[:], psO)
        nc.sync.dma_start(out_d[:], ob[:])

    _compile_with_pinned_act_set(nc)
    return nc


def _compile_with_pinned_act_set(nc):
    """Compile with the ACT table chooser restricted to
    natural_log_exp_and_others (covers every ACT function this kernel uses:
    Exp, Ln, Square, Relu, Copy, Identity). The default first-match chooser
    alternates exp_and_others / natural_log on the Exp<->Ln boundary, paying
    a ~1.3us table reload ~4x per layer. Positions are preserved so
    act_func_set_id stays aligned with act_info.json; the patch is restored
    immediately after compile."""
    import concourse.bacc as bacc_mod
    orig = bacc_mod.get_activation_tables

    def pinned(arch):
        return {name: (funcs if name == "natural_log_exp_and_others" else set())
                for name, funcs in orig(arch).items()}

    bacc_mod.get_activation_tables = pinned
    try:
        nc.compile()
    finally:
        bacc_mod.get_activation_tables = orig


def pe_table():
    pos = np.arange(S, dtype=np.float32)[:, None]
    ie = np.arange(0, C, 2, dtype=np.float32)
    sin = np.sin(pos / 10000.0 ** (2.0 * ie / C))
    cos = np.cos(pos / 10000.0 ** (2.0 * (ie + 1.0) / C))
    pe = np.zeros((S, C), np.float32)
    pe[:, 0::2] = sin
    pe[:, 1::2] = cos
    return pe


_CACHE = {}


def _get_nc(flags):
    if flags not in _CACHE:
        _CACHE[flags] = build_full(
            use_g1=flags[0], use_beta1=flags[1], use_g2=flags[2],
            use_beta2=flags[3], use_bo=flags[4], use_b1=flags[5],
            use_b2=flags[6])
    return _CACHE[flags]


def _bf(a):
    return np.asarray(a).astype(ml_dtypes.bfloat16)


def _relayout(w, inner):
    """[L, n*128, inner] -> [L, 128, n*inner] contiguous per-partition lines."""
    Ln, K, _ = w.shape
    n = K // 128
    return np.ascontiguousarray(
        w.reshape(Ln, n, 128, inner).transpose(0, 2, 1, 3).reshape(Ln, 128, n * inner))


def prep_inputs(x, Wq, Wk, Wv, Wo, bo, g1, beta1, W1, b1, W2, b2, g2, beta2,
                We, be):
    x = np.asarray(x, dtype=np.float32)
    h0 = (np.swapaxes(x, 1, 2) * math.sqrt(C) + pe_table()[None]).astype(np.float32)

    bo, b1, b2 = (np.asarray(a, np.float32) for a in (bo, b1, b2))
    g1, beta1 = (np.asarray(a, np.float32) for a in (g1, beta1))
    g2, beta2 = (np.asarray(a, np.float32) for a in (g2, beta2))
    flags = (bool((g1 != 1).any()), bool(beta1.any()), bool((g2 != 1).any()),
             bool(beta2.any()), bool(bo.any()), bool(b1.any()), bool(b2.any()))

    We_bf = _bf(We)
    base = {"wq": _relayout(_bf(Wq), C), "wk": _relayout(_bf(Wk), C),
            "wv": _relayout(_bf(Wv), C), "wo": _relayout(_bf(Wo), C),
            "w1": _relayout(_bf(W1), FF), "w2": _relayout(_bf(W2), C)}
    names = ("g1", "beta1", "g2", "beta2", "bo", "b1", "b2")
    vals = (g1, beta1, g2, beta2, bo, b1, b2)
    for nm, used, val in zip(names, flags, vals):
        if used:
            base[nm] = val
    in_maps = []
    for c in range(N_CORES):
        m = dict(base)
        m["h0"] = h0[c * SPC:(c + 1) * SPC]
        wsh = We_bf[c * KSH:(c + 1) * KSH]
        m["we"] = np.ascontiguousarray(
            wsh.reshape(NKT // 4, 4, 128, O).transpose(0, 2, 1, 3)
               .reshape(NKT // 4, 128, 4 * O))
        in_maps.append(m)
    return flags, in_maps


def gather_out(results, be):
    """Sum the 8 per-core partials, un-permute rows (the end-layer hT
    columns are ordered (sample, core) = b*8+i while global batch is
    i*SPC+b), and add the bias."""
    out = np.zeros((B, O), np.float32)
    for c in range(N_CORES):
        out += results[c]["out"]
    perm = [(g % SPC) * N_CORES + g // SPC for g in range(B)]
    out = out[perm]
    out += np.asarray(be, np.float32)[None, :]
    return out


def kernel(x, Wq, Wk, Wv, Wo, bo, g1, beta1, W1, b1, W2, b2, g2, beta2, We,
           be, **_unused):
    flags, in_maps = prep_inputs(x, Wq, Wk, Wv, Wo, bo, g1, beta1, W1, b1,
                                 W2, b2, g2, beta2, We, be)
    nc = _get_nc(flags)
    res = run_bass_kernel_spmd(nc, in_maps, list(range(N_CORES)))
    return gather_out(res.results, be)
